# revision 1
# baseline (speedup 1.0000x reference)
"""GRAFF GNN kernel for Trainium2, 8 NeuronCores, SPMD.

Sharding: nodes split 8 ways (12500/core); edges partitioned by destination
node; per-layer AllGather of the bf16 node state; segment-sum as one-hot
scatter matmuls into PSUM per 128-destination window.

Self-contained: hardcodes shapes from the problem spec.
"""
import sys
sys.path.insert(0, "/opt/trn_rl_repo")
import numpy as np
import ml_dtypes

import os
import concourse.bass as bass
import concourse.bacc as bacc
import concourse.tile as tile
from concourse import mybir
from concourse.bass_utils import run_bass_kernel_spmd
from concourse.masks import make_identity
from contextlib import ExitStack

bf16 = ml_dtypes.bfloat16
FP32 = mybir.dt.float32
BF16 = mybir.dt.bfloat16
I32 = mybir.dt.int32
I16 = mybir.dt.int16

NCORE = 8
N = 100000
FIN = 2613
H = 256
NL = 12500            # nodes per core
NW = 98               # windows of 128 dests (last has 84)
NSH = NW * 128        # padded shard rows = 12544
KT = 21               # k-tiles of 128 over FIN (padded to 2688)
FINP = KT * 128
STEP = 0.1
CH = 7                # windows per elementwise chunk (98 = 14*7)
RG = [list(range(NCORE))]


def _build_program(K_w, src_b_val):
    """Build the SPMD Bass program. K_w: list of e-tile counts per window."""
    n_et = int(sum(K_w))
    nc = bacc.Bacc("TRN2", num_devices=NCORE, debug=False)

    # ---- I/O ----
    xt_t = nc.dram_tensor("xt", [FINP, NL], FP32, kind="ExternalInput")
    eidx_t = nc.dram_tensor("eidx", [128, n_et], I32, kind="ExternalInput")
    dn_t = nc.dram_tensor("dn", [128, 2 * n_et], FP32, kind="ExternalInput")
    encw_t = nc.dram_tensor("encw", [128, KT * H], BF16, kind="ExternalInput")
    wp_t = nc.dram_tensor("wp", [128, 2 * H], BF16, kind="ExternalInput")
    l1_t = nc.dram_tensor("l1", [128, 2 * H], BF16, kind="ExternalInput")
    l2_t = nc.dram_tensor("l2", [128, 2 * H], BF16, kind="ExternalInput")
    extw_t = nc.dram_tensor("extw", [128, H], FP32, kind="ExternalInput")
    b1_t = nc.dram_tensor("b1", [128, H], FP32, kind="ExternalInput")
    b2_t = nc.dram_tensor("b2", [128, H], FP32, kind="ExternalInput")
    gb_t = nc.dram_tensor("gb", [1, 2 * H], FP32, kind="ExternalInput")
    y_t = nc.dram_tensor("y", [NL, H], FP32, kind="ExternalOutput")

    # collectives (internal DRAM)
    hsh = [nc.dram_tensor(f"hsh{i}", [NSH, H], BF16, kind="Internal")
           for i in range(4)]
    hfull = [nc.dram_tensor(f"hfull{i}", [NCORE * NSH, H], BF16,
                            kind="Internal", addr_space="Shared")
             for i in range(4)]
    ar_in = nc.dram_tensor("ar_in", [1, 2 * H], FP32, kind="Internal")
    ar_out = nc.dram_tensor("ar_out", [1, 2 * H], FP32, kind="Internal",
                            addr_space="Shared")

    etb = np.concatenate([[0], np.cumsum(K_w)]).astype(int)  # e-tile base per window

    with tile.TileContext(nc) as tc, ExitStack() as ctx:
        const = ctx.enter_context(tc.tile_pool(name="const", bufs=1))
        sb = ctx.enter_context(tc.tile_pool(name="sb", bufs=2))
        msgp = ctx.enter_context(tc.tile_pool(name="msg", bufs=6))
        chp = ctx.enter_context(tc.tile_pool(name="ch", bufs=2))
        chp1 = ctx.enter_context(tc.tile_pool(name="ch1", bufs=1))
        ps = ctx.enter_context(tc.tile_pool(name="ps", bufs=4, space="PSUM"))
        pst = ctx.enter_context(tc.tile_pool(name="pst", bufs=4, space="PSUM"))

        # ---- resident constants ----
        h_sb = const.tile([128, NW * H], BF16)     # node state, [p, w*256+f]
        h0_sb = const.tile([128, NW * H], BF16)    # BN output (h0)
        eidx_sb = const.tile([128, n_et], I32)
        nc.sync.dma_start(out=eidx_sb[:], in_=eidx_t.ap())
        dn_sb = const.tile([128, 2 * n_et], FP32)
        nc.sync.dma_start(out=dn_sb[:], in_=dn_t.ap())
        encw_sb = const.tile([128, KT * H], BF16)
        nc.sync.dma_start(out=encw_sb[:], in_=encw_t.ap())
        wp_sb = const.tile([128, 2 * H], BF16)
        nc.sync.dma_start(out=wp_sb[:], in_=wp_t.ap())
        l1_sb = const.tile([128, 2 * H], BF16)
        nc.sync.dma_start(out=l1_sb[:], in_=l1_t.ap())
        l2_sb = const.tile([128, 2 * H], BF16)
        nc.sync.dma_start(out=l2_sb[:], in_=l2_t.ap())
        extw_sb = const.tile([128, H], FP32)
        nc.sync.dma_start(out=extw_sb[:], in_=extw_t.ap())
        b1_sb = const.tile([128, H], FP32)
        nc.sync.dma_start(out=b1_sb[:], in_=b1_t.ap())
        b2_sb = const.tile([128, H], FP32)
        nc.sync.dma_start(out=b2_sb[:], in_=b2_t.ap())
        gb_sb = const.tile([1, 2 * H], FP32)
        nc.sync.dma_start(out=gb_sb[:], in_=gb_t.ap())

        iota_i = const.tile([128, 128], I16)
        nc.gpsimd.iota(out=iota_i[:], pattern=[[1, 128]], base=0,
                       channel_multiplier=0)
        iota_bf = const.tile([128, 128], BF16)
        nc.vector.tensor_copy(out=iota_bf[:], in_=iota_i[:])
        ident = const.tile([128, 128], BF16)
        make_identity(nc, ident[:])
        ones_col = const.tile([128, 1], FP32)
        nc.vector.memset(ones_col[:], 1.0)
        ones_row = const.tile([1, 128], FP32)
        nc.vector.memset(ones_row[:], 1.0)
        acc_h = const.tile([128, H], FP32)
        nc.vector.memset(acc_h[:], 0.0)
        acc_sq = const.tile([128, H], FP32)
        nc.vector.memset(acc_sq[:], 0.0)
        nc.vector.memset(h_sb[:], 0.0)

        # ================= encoder =================
        for w in range(NW):
            sz = 84 if w == NW - 1 else 128
            xt = sb.tile([128, KT * 128], BF16, tag="xt")
            # one DMA per node-tile: [kt*128+p, w*128+j] -> [p, kt*sz+j], fp32->bf16
            src = xt_t.ap().rearrange("(kt p) n -> p kt n", p=128)
            nc.gpsimd.dma_start(
                out=xt[:].rearrange("p (kt n) -> p kt n", kt=KT)[:, :, :sz],
                in_=src[:, :, w * 128: w * 128 + sz])
            ph = ps.tile([128, H], FP32, tag="mm", space="PSUM")
            for kt in range(KT):
                nc.tensor.matmul(
                    ph[:sz, :], lhsT=xt[:, kt * 128: kt * 128 + sz],
                    rhs=encw_sb[:, kt * H: (kt + 1) * H],
                    start=(kt == 0), stop=(kt == KT - 1))
            sq = sb.tile([128, H], FP32, tag="sq")
            nc.scalar.square(sq[:sz, :], ph[:sz, :])
            nc.vector.tensor_add(acc_h[:sz, :], acc_h[:sz, :], ph[:sz, :])
            nc.vector.tensor_add(acc_sq[:sz, :], acc_sq[:sz, :], sq[:sz, :])
            nc.vector.tensor_copy(out=h_sb[:sz, w * H: (w + 1) * H],
                                  in_=ph[:sz, :])

        # ================= batch norm =================
        psum_s = ps.tile([1, H], FP32, tag="mm", space="PSUM")
        nc.tensor.matmul(psum_s[:], lhsT=ones_col[:, :1], rhs=acc_h[:],
                         start=True, stop=True)
        psum_q = ps.tile([1, H], FP32, tag="mm", space="PSUM")
        nc.tensor.matmul(psum_q[:], lhsT=ones_col[:, :1], rhs=acc_sq[:],
                         start=True, stop=True)
        st_sb = sb.tile([1, 2 * H], FP32, tag="st")
        nc.vector.tensor_copy(out=st_sb[:, :H], in_=psum_s[:])
        nc.vector.tensor_copy(out=st_sb[:, H:], in_=psum_q[:])
        nc.gpsimd.dma_start(out=ar_in.ap(), in_=st_sb[:])
        if os.environ.get("K_NOAG"):
            nc.sync.dma_start(out=ar_out.ap(), in_=ar_in.ap())
        else:
            nc.gpsimd.collective_compute(
                "AllReduce", mybir.AluOpType.add, replica_groups=RG,
                ins=[ar_in.ap()], outs=[ar_out.ap()])
        sg = sb.tile([1, 2 * H], FP32, tag="sg")
        nc.sync.dma_start(out=sg[:], in_=ar_out.ap())

        ss = sb.tile([1, 2 * H], FP32, tag="ss")   # [scale | shift]
        mean = sb.tile([1, H], FP32, tag="mean")
        nc.vector.tensor_scalar(out=mean[:], in0=sg[:, :H], scalar1=1.0 / N,
                                scalar2=None, op0=mybir.AluOpType.mult)
        var = sb.tile([1, H], FP32, tag="var")
        nc.vector.tensor_scalar(out=var[:], in0=sg[:, H:], scalar1=1.0 / N,
                                scalar2=None, op0=mybir.AluOpType.mult)
        msq = sb.tile([1, H], FP32, tag="msq")
        nc.vector.tensor_mul(msq[:], mean[:], mean[:])
        nc.vector.tensor_sub(var[:], var[:], msq[:])
        nc.vector.tensor_scalar(out=var[:], in0=var[:], scalar1=1e-5,
                                scalar2=None, op0=mybir.AluOpType.add)
        std = sb.tile([1, H], FP32, tag="stdv")
        nc.scalar.sqrt(std[:], var[:])
        inv = sb.tile([1, H], FP32, tag="inv")
        nc.vector.reciprocal(inv[:], std[:])
        nc.vector.tensor_mul(ss[:, :H], inv[:], gb_sb[:, :H])          # scale
        t0 = sb.tile([1, H], FP32, tag="t0s")
        nc.vector.tensor_mul(t0[:], mean[:], ss[:, :H])
        nc.vector.tensor_sub(ss[:, H:], gb_sb[:, H:], t0[:])            # shift
        pb = ps.tile([128, 2 * H], FP32, tag="mm", space="PSUM")
        nc.tensor.matmul(pb[:], lhsT=ones_row[:1, :], rhs=ss[:1, :],
                         start=True, stop=True)
        srow = sb.tile([128, 2 * H], FP32, tag="srow")
        nc.vector.tensor_copy(out=srow[:], in_=pb[:])

        for w in range(NW):
            sz = 84 if w == NW - 1 else 128
            wsl = slice(w * H, (w + 1) * H)
            tb = sb.tile([128, H], FP32, tag="sq")
            nc.vector.tensor_mul(tb[:sz], h_sb[:sz, wsl], srow[:sz, :H])
            nc.vector.tensor_add(h_sb[:sz, wsl], tb[:sz], srow[:sz, H:])
        nc.vector.tensor_copy(out=h0_sb[:], in_=h_sb[:])

        def stage_and_allgather(li):
            nc.sync.dma_start(
                out=hsh[li].ap().rearrange("(w p) f -> p w f", p=128),
                in_=h_sb[:].rearrange("p (w f) -> p w f", f=H))
            if os.environ.get("K_NOAG"):
                nc.sync.dma_start(
                    out=hfull[li].ap()[:NSH, :],
                    in_=hsh[li].ap())
            else:
                nc.gpsimd.collective_compute(
                    "AllGather", mybir.AluOpType.bypass, replica_groups=RG,
                    ins=[hsh[li].ap()], outs=[hfull[li].ap()])

        stage_and_allgather(0)

        # ================= GNN layers =================
        _nlayers = int(os.environ.get("K_NLAYERS", "4"))
        for li in range(_nlayers):
            hf = hfull[li].ap()
            for wc in range(NW // CH):        # 14 chunks of 7 windows
                u_ch = chp.tile([128, CH * H], FP32, tag="uch")
                for wi in range(CH):
                    w = wc * CH + wi
                    pagg = ps.tile([128, H], FP32, tag="mm", space="PSUM")
                    for t in range(K_w[w]):
                        et = int(etb[w]) + t
                        msg = msgp.tile([128, H], BF16, tag="msg")
                        if os.environ.get("K_NOGATHER"):
                            nc.sync.dma_start(out=msg[:],
                                              in_=hf[et * 128:(et + 1) * 128, :])
                        else:
                            nc.gpsimd.indirect_dma_start(
                                out=msg[:], out_offset=None, in_=hf,
                                in_offset=bass.IndirectOffsetOnAxis(
                                    ap=eidx_sb[:, et:et + 1], axis=0))
                        S0 = sb.tile([128, 128], BF16, tag="S0")
                        nc.vector.tensor_tensor(
                            out=S0[:],
                            in0=dn_sb[:, 2 * et:2 * et + 1].to_broadcast([128, 128]),
                            in1=iota_bf[:], op=mybir.AluOpType.is_equal)
                        S = sb.tile([128, 128], BF16, tag="S")
                        nc.vector.tensor_tensor(
                            out=S[:], in0=S0[:],
                            in1=dn_sb[:, 2 * et + 1:2 * et + 2].to_broadcast([128, 128]),
                            op=mybir.AluOpType.mult)
                        nc.tensor.matmul(pagg[:], lhsT=S[:], rhs=msg[:],
                                         start=(t == 0), stop=(t == K_w[w] - 1))
                    agg_bf = sb.tile([128, H], BF16, tag="aggbf")
                    nc.vector.tensor_copy(out=agg_bf[:], in_=pagg[:])
                    pt1 = pst.tile([128, 128], BF16, tag="tp", space="PSUM")
                    nc.tensor.transpose(out=pt1[:], in_=agg_bf[:, :128],
                                        identity=ident[:])
                    pt2 = pst.tile([128, 128], BF16, tag="tp", space="PSUM")
                    nc.tensor.transpose(out=pt2[:], in_=agg_bf[:, 128:],
                                        identity=ident[:])
                    aggT = sb.tile([128, H], BF16, tag="aggT")
                    nc.vector.tensor_copy(out=aggT[:, :128], in_=pt1[:])
                    nc.vector.tensor_copy(out=aggT[:, 128:], in_=pt2[:])
                    pupd = ps.tile([128, H], FP32, tag="mm", space="PSUM")
                    nc.tensor.matmul(pupd[:], lhsT=aggT[:, :128],
                                     rhs=wp_sb[:, :H], start=True, stop=False)
                    nc.tensor.matmul(pupd[:], lhsT=aggT[:, 128:],
                                     rhs=wp_sb[:, H:], start=False, stop=True)
                    # u = pupd - h*extw - src_b*h0
                    wsl = slice(w * H, (w + 1) * H)
                    usl = slice(wi * H, (wi + 1) * H)
                    t1 = sb.tile([128, H], FP32, tag="sq")
                    nc.vector.tensor_mul(t1[:], h_sb[:, wsl], extw_sb[:])
                    nc.vector.scalar_tensor_tensor(
                        out=u_ch[:, usl], in0=h0_sb[:, wsl], scalar=-src_b_val,
                        in1=pupd[:], op0=mybir.AluOpType.mult,
                        op1=mybir.AluOpType.add)
                    nc.vector.tensor_sub(u_ch[:, usl], u_ch[:, usl], t1[:])
                # chunk: h += 0.1*elu(u) ; elu = relu(u) - relu(1-exp(u))
                csl = slice(wc * CH * H, (wc + 1) * CH * H)
                e_ch = chp1.tile([128, CH * H], BF16, tag="ech")
                nc.scalar.activation(e_ch[:], u_ch[:],
                                     mybir.ActivationFunctionType.Exp)
                a_ch = chp1.tile([128, CH * H], BF16, tag="ach")
                nc.scalar.activation(a_ch[:], u_ch[:],
                                     mybir.ActivationFunctionType.Relu,
                                     scale=STEP)
                nc.vector.tensor_scalar(out=e_ch[:], in0=e_ch[:], scalar1=-1.0,
                                        scalar2=1.0, op0=mybir.AluOpType.mult,
                                        op1=mybir.AluOpType.add)
                nc.vector.tensor_scalar(out=e_ch[:], in0=e_ch[:], scalar1=0.0,
                                        scalar2=STEP, op0=mybir.AluOpType.max,
                                        op1=mybir.AluOpType.mult)
                nc.vector.tensor_sub(a_ch[:], a_ch[:], e_ch[:])
                nc.vector.tensor_add(h_sb[:, csl], h_sb[:, csl], a_ch[:])
            if li < 3:
                stage_and_allgather(li + 1)

        # ================= MLP =================
        _mlp_chunks = 0 if os.environ.get("K_NOMLP") else NW // CH
        for wc in range(_mlp_chunks):
            u_ch = chp.tile([128, CH * H], FP32, tag="uch")
            for wi in range(CH):
                w = wc * CH + wi
                wsl = slice(w * H, (w + 1) * H)
                usl = slice(wi * H, (wi + 1) * H)
                pt1 = pst.tile([128, 128], BF16, tag="tp", space="PSUM")
                nc.tensor.transpose(out=pt1[:], in_=h_sb[:, w * H: w * H + 128],
                                    identity=ident[:])
                pt2 = pst.tile([128, 128], BF16, tag="tp", space="PSUM")
                nc.tensor.transpose(out=pt2[:], in_=h_sb[:, w * H + 128: (w + 1) * H],
                                    identity=ident[:])
                hT = sb.tile([128, H], BF16, tag="aggT")
                nc.vector.tensor_copy(out=hT[:, :128], in_=pt1[:])
                nc.vector.tensor_copy(out=hT[:, 128:], in_=pt2[:])
                pg = ps.tile([128, H], FP32, tag="mm", space="PSUM")
                nc.tensor.matmul(pg[:], lhsT=hT[:, :128], rhs=l1_sb[:, :H],
                                 start=True, stop=False)
                nc.tensor.matmul(pg[:], lhsT=hT[:, 128:], rhs=l1_sb[:, H:],
                                 start=False, stop=True)
                nc.vector.tensor_add(u_ch[:, usl], pg[:], b1_sb[:])
            # g = elu(u) = relu(u) - relu(1-exp(u))
            e_ch = chp1.tile([128, CH * H], BF16, tag="ech")
            nc.scalar.activation(e_ch[:], u_ch[:],
                                 mybir.ActivationFunctionType.Exp)
            g_ch = chp1.tile([128, CH * H], BF16, tag="ach")
            nc.scalar.activation(g_ch[:], u_ch[:],
                                 mybir.ActivationFunctionType.Relu)
            nc.vector.tensor_scalar(out=e_ch[:], in0=e_ch[:], scalar1=-1.0,
                                    scalar2=1.0, op0=mybir.AluOpType.mult,
                                    op1=mybir.AluOpType.add)
            nc.vector.tensor_scalar(out=e_ch[:], in0=e_ch[:], scalar1=0.0,
                                    scalar2=None, op0=mybir.AluOpType.max)
            nc.vector.tensor_sub(g_ch[:], g_ch[:], e_ch[:])
            for wi in range(CH):
                w = wc * CH + wi
                sz = 84 if w == NW - 1 else 128
                usl128 = slice(wi * H, wi * H + 128)
                usl256 = slice(wi * H + 128, (wi + 1) * H)
                pt1 = pst.tile([128, 128], BF16, tag="tp", space="PSUM")
                nc.tensor.transpose(out=pt1[:], in_=g_ch[:, usl128],
                                    identity=ident[:])
                pt2 = pst.tile([128, 128], BF16, tag="tp", space="PSUM")
                nc.tensor.transpose(out=pt2[:], in_=g_ch[:, usl256],
                                    identity=ident[:])
                gT = sb.tile([128, H], BF16, tag="aggT")
                nc.vector.tensor_copy(out=gT[:, :128], in_=pt1[:])
                nc.vector.tensor_copy(out=gT[:, 128:], in_=pt2[:])
                py = ps.tile([128, H], FP32, tag="mm", space="PSUM")
                nc.tensor.matmul(py[:], lhsT=gT[:, :128], rhs=l2_sb[:, :H],
                                 start=True, stop=False)
                nc.tensor.matmul(py[:], lhsT=gT[:, 128:], rhs=l2_sb[:, H:],
                                 start=False, stop=True)
                y_sb = sb.tile([128, H], FP32, tag="sq")
                nc.vector.tensor_add(y_sb[:], py[:], b2_sb[:])
                nc.sync.dma_start(out=y_t.ap()[w * 128: w * 128 + sz, :],
                                  in_=y_sb[:sz, :])

    nc.compile()
    return nc


def _host_prep(x, edge_index, enc_w, bn_gamma, bn_beta, ext_w, src_b, pw_W,
               lin1_w, lin1_b, lin2_w, lin2_b):
    x = np.asarray(x, dtype=np.float32)
    ei = np.asarray(edge_index)
    row = ei[0].astype(np.int64)
    col = ei[1].astype(np.int64)

    # pairwise matrix
    pw = np.asarray(pw_W, dtype=np.float32)
    W0 = np.triu(pw[:, :-2], k=1)
    W0 = W0 + W0.T
    Wp = W0 + np.diag(pw[:, -2] * np.abs(W0).sum(1) + pw[:, -1])

    deg = np.bincount(col, minlength=N).astype(np.float32)
    dinv = np.where(deg > 0, deg ** -0.5, 0.0).astype(np.float32)
    nrm = (dinv[row] * dinv[col]).astype(np.float32)

    order = np.argsort(col, kind="stable")
    row_s, col_s, nrm_s = row[order], col[order], nrm[order]
    core_s = col_s // NL
    wloc_s = (col_s % NL) // 128
    dloc_s = (col_s % NL - wloc_s * 128).astype(np.float32)
    # padded global row index (hfull has NSH rows per core)
    rowg_s = ((row_s // NL) * NSH + row_s % NL).astype(np.int32)

    counts = np.zeros((NCORE, NW), dtype=np.int64)
    np.add.at(counts, (core_s, wloc_s), 1)
    K_w = np.maximum(1, np.ceil(counts.max(0) / 128).astype(np.int64))
    n_et = int(K_w.sum())
    etb = np.concatenate([[0], np.cumsum(K_w)]).astype(np.int64)

    # slot of each edge inside its (core, window) block
    wid = core_s * NW + wloc_s
    start_of_block = np.zeros(NCORE * NW + 1, dtype=np.int64)
    np.add.at(start_of_block[1:], wid, 1)
    start_of_block = np.cumsum(start_of_block)
    rank = np.arange(len(col_s)) - start_of_block[wid]
    slot = etb[wloc_s] * 128 + rank  # within-core padded edge slot

    eidxs, dns = [], []
    for c in range(NCORE):
        m = core_s == c
        eidx_pad = np.zeros(n_et * 128, dtype=np.int32)
        nrm_pad = np.zeros(n_et * 128, dtype=np.float32)
        dloc_pad = np.full(n_et * 128, -1000.0, dtype=np.float32)
        eidx_pad[slot[m]] = rowg_s[m]
        nrm_pad[slot[m]] = nrm_s[m]
        dloc_pad[slot[m]] = dloc_s[m]
        eidx_T = eidx_pad.reshape(n_et, 128).T.copy()
        dn = np.empty((128, 2 * n_et), dtype=np.float32)
        dn[:, 0::2] = dloc_pad.reshape(n_et, 128).T
        dn[:, 1::2] = nrm_pad.reshape(n_et, 128).T
        eidxs.append(eidx_T)
        dns.append(dn)

    # per-core padded x^T
    xts = []
    for c in range(NCORE):
        xt = np.zeros((FINP, NL), dtype=np.float32)
        xt[:FIN] = np.ascontiguousarray(x[c * NL:(c + 1) * NL].T)
        xts.append(xt)

    def ktile_layout(mat_T, nk):  # mat_T [nk*128, H] -> [128, nk*H]
        out = np.zeros((128, nk * H), dtype=bf16)
        for kt in range(nk):
            blk = mat_T[kt * 128:(kt + 1) * 128]
            out[:blk.shape[0], kt * H:kt * H + blk.shape[1]] = blk.astype(bf16)
        return out

    enc_wT = np.zeros((FINP, H), dtype=np.float32)
    enc_wT[:FIN] = np.asarray(enc_w, np.float32).T
    encw_h = ktile_layout(enc_wT, KT)
    wp_h = ktile_layout(Wp, 2)                      # symmetric: Wp rows
    l1_h = ktile_layout(np.asarray(lin1_w, np.float32).T, 2)
    l2_h = ktile_layout(np.asarray(lin2_w, np.float32).T, 2)
    extw_h = np.tile(np.asarray(ext_w, np.float32).reshape(1, H), (128, 1))
    b1_h = np.tile(np.asarray(lin1_b, np.float32).reshape(1, H), (128, 1))
    b2_h = np.tile(np.asarray(lin2_b, np.float32).reshape(1, H), (128, 1))
    gb_h = np.concatenate([np.asarray(bn_gamma, np.float32),
                           np.asarray(bn_beta, np.float32)]).reshape(1, 2 * H)

    in_maps = []
    for c in range(NCORE):
        in_maps.append({
            "xt": xts[c], "eidx": eidxs[c], "dn": dns[c],
            "encw": encw_h, "wp": wp_h, "l1": l1_h, "l2": l2_h,
            "extw": extw_h, "b1": b1_h, "b2": b2_h, "gb": gb_h,
        })
    return K_w, float(np.asarray(src_b).reshape(-1)[0]), in_maps


def _run(inputs, trace=False):
    K_w, src_b_val, in_maps = _host_prep(**inputs)
    nc = _build_program(list(K_w), src_b_val)
    res = run_bass_kernel_spmd(nc, in_maps, core_ids=list(range(NCORE)),
                               trace=trace)
    y = np.concatenate([res.results[c]["y"] for c in range(NCORE)], 0)
    return y.astype(np.float32), res


def kernel(**inputs):
    y, _ = _run(inputs, trace=False)
    return y


def _timed_run(inputs, n_iter=3):
    """Correctness + warm timing: jit once, device_put inputs, time execs."""
    import time as _time
    import jax
    from jax.sharding import Mesh, PartitionSpec, NamedSharding
    from jax.experimental.shard_map import shard_map
    from concourse import bass2jax, mybir as _mb

    K_w, src_b_val, in_maps = _host_prep(**inputs)
    nc = _build_program(list(K_w), src_b_val)
    bass2jax.install_neuronx_cc_hook()

    partition_name = (nc.partition_id_tensor.name
                      if nc.partition_id_tensor else None)
    in_names, out_names, out_avals, zero_outs = [], [], [], []
    for alloc in nc.m.functions[0].allocations:
        if not isinstance(alloc, _mb.MemoryLocationSet):
            continue
        name = alloc.memorylocations[0].name
        if alloc.kind == "ExternalInput":
            if name != partition_name:
                in_names.append(name)
        elif alloc.kind == "ExternalOutput":
            out_names.append(name)
            shape = tuple(alloc.tensor_shape)
            dtype = _mb.dt.np(alloc.dtype)
            out_avals.append(jax.core.ShapedArray(shape, dtype))
            zero_outs.append(np.zeros(shape, dtype))
    n_params = len(in_names)
    n_outs = len(out_avals)
    in_names_all = in_names + out_names
    if partition_name is not None:
        in_names_all.append(partition_name)
    donate = tuple(range(n_params, n_params + n_outs))

    def _body(*args):
        operands = list(args)
        if partition_name is not None:
            operands.append(bass2jax.partition_id_tensor())
        outs = bass2jax._bass_exec_p.bind(
            *operands, out_avals=tuple(out_avals),
            in_names=tuple(in_names_all), out_names=tuple(out_names),
            lowering_input_output_aliases=(),
            sim_require_finite=True, sim_require_nnan=True, nc=nc)
        return tuple(outs)

    devices = jax.devices()[:NCORE]
    mesh = Mesh(np.asarray(devices), ("core",))
    sharded = jax.jit(
        shard_map(_body, mesh=mesh,
                  in_specs=(PartitionSpec("core"),) * (n_params + n_outs),
                  out_specs=(PartitionSpec("core"),) * n_outs,
                  check_rep=False),
        donate_argnums=donate, keep_unused=True)

    sh = NamedSharding(mesh, PartitionSpec("core"))
    concat_in = [
        jax.device_put(
            np.concatenate([np.asarray(in_maps[c][n]) for c in range(NCORE)], 0),
            sh)
        for n in in_names]
    times = []
    out_arrs = None
    for it in range(n_iter):
        concat_zeros = [
            jax.device_put(np.zeros((NCORE * z.shape[0], *z.shape[1:]), z.dtype), sh)
            for z in zero_outs]
        for z in concat_zeros:
            z.block_until_ready()
        t0 = _time.perf_counter()
        out_arrs = sharded(*concat_in, *concat_zeros)
        for o in out_arrs:
            o.block_until_ready()
        times.append(_time.perf_counter() - t0)
    y_full = np.asarray(out_arrs[out_names.index("y")])
    y = y_full.reshape(NCORE, NL, H).reshape(NCORE * NL, H)
    return y.astype(np.float32), times



# revision 43
# speedup vs baseline: 27.4168x; 27.4168x over previous
"""GRAFF GNN kernel for Trainium2, 8 NeuronCores, SPMD.

Sharding: nodes split 8 ways (12500/core); edges partitioned by destination
node; per-layer AllGather of the bf16 node state; segment-sum as one-hot
scatter matmuls into PSUM per 128-destination window.

Self-contained: hardcodes shapes from the problem spec.
"""
import sys
sys.path.insert(0, "/opt/trn_rl_repo")
import numpy as np
import ml_dtypes

import os
import concourse.bass as bass
import concourse.bacc as bacc
import concourse.tile as tile
from concourse import mybir
from concourse.bass_utils import run_bass_kernel_spmd
from concourse.masks import make_identity
from contextlib import ExitStack

bf16 = ml_dtypes.bfloat16
FP32 = mybir.dt.float32
BF16 = mybir.dt.bfloat16
I32 = mybir.dt.int32
I16 = mybir.dt.int16

NCORE = 8
N = 100000
FIN = 2613
H = 256
NL = 12500            # nodes per core
NW = 98               # windows of 128 dests (last has 84)
NSH = NW * 128        # padded shard rows = 12544
KT = 21               # k-tiles of 128 over FIN (padded to 2688)
FINP = KT * 128
STEP = 0.1
CH = 7                # windows per elementwise chunk (98 = 14*7)
RG = [list(range(NCORE))]


def _build_program(meta):
    """Build the SPMD Bass program.

    meta: dict with
      n_et: total e-tile count
      idx_cols: total int16 idx columns
      calls: per window-chunk, list of (bucket, n_tiles, col_off, tile_start)
      wtiles: per window, list of global tile ids (in matmul order)
      chunk_t0: per window-chunk, first global tile id
      src_b: float
    """
    n_et = meta["n_et"]
    idx_cols = meta["idx_cols"]
    src_b_val = meta["src_b"]
    nc = bacc.Bacc("TRN2", num_devices=NCORE, debug=False)

    # ---- I/O ----
    xt_t = nc.dram_tensor("xt", [FINP, NSH], BF16, kind="ExternalInput")
    idxw_t = nc.dram_tensor("idxw", [128, idx_cols], I16, kind="ExternalInput")
    sm_t = nc.dram_tensor("sm", [128, n_et * 128], BF16, kind="ExternalInput")
    encw_t = nc.dram_tensor("encw", [128, KT * H], BF16, kind="ExternalInput")
    wp_t = nc.dram_tensor("wp", [128, 2 * H], BF16, kind="ExternalInput")
    l1_t = nc.dram_tensor("l1", [128, 2 * H], BF16, kind="ExternalInput")
    l2_t = nc.dram_tensor("l2", [128, 2 * H], BF16, kind="ExternalInput")
    extw_t = nc.dram_tensor("extw", [128, H], FP32, kind="ExternalInput")
    b1_t = nc.dram_tensor("b1", [128, H], FP32, kind="ExternalInput")
    b2_t = nc.dram_tensor("b2", [128, H], FP32, kind="ExternalInput")
    gb_t = nc.dram_tensor("gb", [1, 2 * H], FP32, kind="ExternalInput")
    sdeg_t = nc.dram_tensor("sdeg", [128, NW], FP32, kind="ExternalInput")
    y_t = nc.dram_tensor("y", [NL, H], FP32, kind="ExternalOutput")

    # collectives (internal DRAM), split into window quarters
    QW = [(0, 25), (25, 25), (50, 24), (74, 24)]   # (w0, nw) per quarter
    hshQ = [[nc.dram_tensor(f"hsh{i}q{q}", [nw * 128, H], BF16,
                            kind="Internal")
             for q, (w0, nw) in enumerate(QW)] for i in range(4)]
    hfullQ = [[nc.dram_tensor(f"hfull{i}q{q}", [NCORE * nw * 128, H], BF16,
                              kind="Internal", addr_space="Shared")
               for q, (w0, nw) in enumerate(QW)] for i in range(4)]
    ar_in = nc.dram_tensor("ar_in", [4, 128], FP32, kind="Internal")
    ar_out = nc.dram_tensor("ar_out", [4, 128], FP32, kind="Internal",
                            addr_space="Shared")
    h0d_t = nc.dram_tensor("h0d", [128, NW * H], BF16, kind="Internal")

    with tile.TileContext(nc) as tc, ExitStack() as ctx:
        const = ctx.enter_context(tc.tile_pool(name="const", bufs=1))
        sb = ctx.enter_context(tc.tile_pool(name="sb", bufs=2))
        ps = ctx.enter_context(tc.tile_pool(name="ps", bufs=4, space="PSUM"))
        pst = ctx.enter_context(tc.tile_pool(name="pst", bufs=4, space="PSUM"))

        # ---- resident constants ----
        h_sb = const.tile([128, NW * H], BF16)     # node state, [p, w*256+f]
        idxw_sb = const.tile([128, idx_cols], I16)
        nc.sync.dma_start(out=idxw_sb[:], in_=idxw_t.ap())
        encw_sb = const.tile([128, KT * H], BF16)
        nc.sync.dma_start(out=encw_sb[:], in_=encw_t.ap())
        wp_sb = const.tile([128, 2 * H], BF16)
        nc.sync.dma_start(out=wp_sb[:], in_=wp_t.ap())
        l1_sb = const.tile([128, 2 * H], BF16)
        nc.sync.dma_start(out=l1_sb[:], in_=l1_t.ap())
        l2_sb = const.tile([128, 2 * H], BF16)
        nc.sync.dma_start(out=l2_sb[:], in_=l2_t.ap())
        extw_sb = const.tile([128, H], FP32)
        nc.sync.dma_start(out=extw_sb[:], in_=extw_t.ap())
        b1_sb = const.tile([128, H], FP32)
        nc.sync.dma_start(out=b1_sb[:], in_=b1_t.ap())
        b2_sb = const.tile([128, H], FP32)
        nc.sync.dma_start(out=b2_sb[:], in_=b2_t.ap())
        gb_sb = const.tile([1, 2 * H], FP32)
        nc.sync.dma_start(out=gb_sb[:], in_=gb_t.ap())
        sdeg_sb = const.tile([128, NW], FP32)
        nc.sync.dma_start(out=sdeg_sb[:], in_=sdeg_t.ap())

        ident = const.tile([128, 128], BF16)
        make_identity(nc, ident[:])
        ones_row = const.tile([1, 128], FP32)
        nc.vector.memset(ones_row[:], 1.0)
        accs = const.tile([128, 4], FP32)   # [sum,sq] x fb, feature-major
        nc.vector.memset(accs[:], 0.0)
        nc.vector.memset(h_sb[:], 0.0)

        def stage_and_allgather(li, q):
            w0, nw = QW[q]
            nc.sync.dma_start(
                out=hshQ[li][q].ap().rearrange("(w p) f -> p w f", p=128),
                in_=h_sb[:, w0 * H:(w0 + nw) * H]
                .rearrange("p (w f) -> p w f", f=H))
            nc.gpsimd.collective_compute(
                "AllGather", mybir.AluOpType.bypass, replica_groups=RG,
                ins=[hshQ[li][q].ap()], outs=[hfullQ[li][q].ap()])

        # ========= encoder: feature-major blocks, 512-node free =========
        NBL = [(i * 512, 512) for i in range(24)] + [(12288, 256)]
        src_x = xt_t.ap().rearrange("(kt p) n -> p kt n", p=128)
        with tc.tile_pool(name="xtp", bufs=2) as xtp:
            for (n0, nlen) in NBL:
                xtb = xtp.tile([128, KT * 512], BF16, tag="xt")
                nc.sync.dma_start(
                    out=xtb[:].rearrange("p (kt n) -> p kt n", n=512)
                    [:, :, :nlen],
                    in_=src_x[:, :, n0:n0 + nlen])
                for fb in range(2):
                    pe = ps.tile([128, 512], FP32, tag="mm", space="PSUM")
                    f0 = fb * 128
                    for kt in range(KT):
                        nc.tensor.matmul(
                            pe[:, :nlen],
                            lhsT=encw_sb[:, kt * H + f0: kt * H + f0 + 128],
                            rhs=xtb[:, kt * 512: kt * 512 + nlen],
                            start=(kt == 0), stop=(kt == KT - 1))
                    hbf = sb.tile([128, 512], BF16, tag="hbf")
                    nc.vector.tensor_copy(out=hbf[:, :nlen], in_=pe[:, :nlen])
                    sq = sb.tile([128, 512], FP32, tag="sq")
                    nc.scalar.square(sq[:, :nlen], pe[:, :nlen])
                    red = sb.tile([128, 2], FP32, tag="red")
                    nc.vector.reduce_sum(out=red[:, 0:1], in_=pe[:, :nlen],
                                         axis=mybir.AxisListType.X)
                    nc.vector.reduce_sum(out=red[:, 1:2], in_=sq[:, :nlen],
                                         axis=mybir.AxisListType.X)
                    nc.vector.tensor_add(accs[:, 2 * fb:2 * fb + 2],
                                         accs[:, 2 * fb:2 * fb + 2], red[:])
                    for j in range(nlen // 128):
                        w = n0 // 128 + j
                        ptE = pst.tile([128, 128], BF16, tag="tp",
                                       space="PSUM")
                        nc.tensor.transpose(
                            out=ptE[:], in_=hbf[:, j * 128:(j + 1) * 128],
                            identity=ident[:])
                        nc.vector.tensor_copy(
                            out=h_sb[:, w * H + f0: w * H + f0 + 128],
                            in_=ptE[:])
                # layer-0 AllGather ships RAW (pre-BN) h
                if n0 == 3072:
                    stage_and_allgather(0, 0)
                elif n0 == 6144:
                    stage_and_allgather(0, 1)
                elif n0 == 9216:
                    stage_and_allgather(0, 2)

        msgp = ctx.enter_context(tc.tile_pool(name="msg", bufs=2))
        chp = ctx.enter_context(tc.tile_pool(name="ch", bufs=2))
        chp1 = ctx.enter_context(tc.tile_pool(name="ch1", bufs=1))

        # ================= batch norm =================
        nc.sync.dma_start(out=ar_in.ap().rearrange("c p -> p c"),
                          in_=accs[:])
        if os.environ.get("K_NOAG"):
            nc.sync.dma_start(out=ar_out.ap(), in_=ar_in.ap())
        else:
            nc.gpsimd.collective_compute(
                "AllReduce", mybir.AluOpType.add, replica_groups=RG,
                ins=[ar_in.ap()], outs=[ar_out.ap()])
        stage_and_allgather(0, 3)
        sg = sb.tile([1, 2 * H], FP32, tag="sg", bufs=1)
        for fb in range(2):
            nc.sync.dma_start(out=sg[:, fb * 128:(fb + 1) * 128],
                              in_=ar_out.ap()[2 * fb:2 * fb + 1, :])
            nc.sync.dma_start(out=sg[:, H + fb * 128:H + (fb + 1) * 128],
                              in_=ar_out.ap()[2 * fb + 1:2 * fb + 2, :])

        ss = sb.tile([1, 2 * H], FP32, tag="ss", bufs=1)   # [scale | shift]
        mean = sb.tile([1, H], FP32, tag="mean", bufs=1)
        nc.vector.tensor_scalar(out=mean[:], in0=sg[:, :H], scalar1=1.0 / N,
                                scalar2=None, op0=mybir.AluOpType.mult)
        var = sb.tile([1, H], FP32, tag="var", bufs=1)
        nc.vector.tensor_scalar(out=var[:], in0=sg[:, H:], scalar1=1.0 / N,
                                scalar2=None, op0=mybir.AluOpType.mult)
        msq = sb.tile([1, H], FP32, tag="msq", bufs=1)
        nc.vector.tensor_mul(msq[:], mean[:], mean[:])
        nc.vector.tensor_sub(var[:], var[:], msq[:])
        nc.vector.tensor_scalar(out=var[:], in0=var[:], scalar1=1e-5,
                                scalar2=None, op0=mybir.AluOpType.add)
        std = sb.tile([1, H], FP32, tag="stdv", bufs=1)
        nc.scalar.sqrt(std[:], var[:])
        inv = sb.tile([1, H], FP32, tag="inv", bufs=1)
        nc.vector.reciprocal(inv[:], std[:])
        nc.vector.tensor_mul(ss[:, :H], inv[:], gb_sb[:, :H])          # scale
        t0 = sb.tile([1, H], FP32, tag="t0s", bufs=1)
        nc.vector.tensor_mul(t0[:], mean[:], ss[:, :H])
        nc.vector.tensor_sub(ss[:, H:], gb_sb[:, H:], t0[:])            # shift
        pb = ps.tile([128, 2 * H], FP32, tag="mm", space="PSUM")
        nc.tensor.matmul(pb[:], lhsT=ones_row[:1, :], rhs=ss[:1, :],
                         start=True, stop=True)
        srow = sb.tile([128, 2 * H], FP32, tag="srow", bufs=1)
        nc.vector.tensor_copy(out=srow[:], in_=pb[:])

        for w in range(NW):
            sz = 84 if w == NW - 1 else 128
            wsl = slice(w * H, (w + 1) * H)
            tb = sb.tile([128, H], FP32, tag="sq")
            nc.vector.tensor_mul(tb[:sz], h_sb[:sz, wsl], srow[:sz, :H])
            nc.vector.tensor_add(h_sb[:sz, wsl], tb[:sz], srow[:sz, H:])
        nc.sync.dma_start(out=h0d_t.ap(), in_=h_sb[:])
        extw0_sb = const.tile([128, H], FP32)
        nc.vector.tensor_scalar(out=extw0_sb[:], in0=extw_sb[:],
                                scalar1=src_b_val, scalar2=None,
                                op0=mybir.AluOpType.add)

        # ================= GNN layers =================
        _nlayers = int(os.environ.get("K_NLAYERS", "4"))
        for li in range(_nlayers):
            hfq = [hfullQ[li][q].ap() for q in range(4)]
            for wc in range(NW // CH):        # 14 chunks of 7 windows
                ct0 = meta["chunk_t0"][wc]
                ctn = meta["chunk_t0"][wc + 1] - ct0
                # batched gathers: one dma_gather per (group x <=8 tiles)
                mga = msgp.tile([128, ctn * H], BF16, tag="mg", bufs=3)
                for (g, nt, co, tg) in meta["calls"][wc]:
                    nc.gpsimd.dma_gather(
                        mga[:, (tg - ct0) * H:(tg - ct0 + nt) * H]
                        .rearrange("p (t h) -> p t h", h=H),
                        hfq[g],
                        idxw_sb[:, co:co + 8 * nt],
                        128 * nt, 128 * nt, H)
                # streamed host-precomputed S for the whole chunk
                S = msgp.tile([128, ctn * 128], BF16, tag="S")
                nc.sync.dma_start(
                    out=S[:], in_=sm_t.ap()[:, ct0 * 128:(ct0 + ctn) * 128])

                u_ch = chp.tile([128, CH * H], BF16, tag="uch")
                for wi in range(CH):
                    w = wc * CH + wi
                    tiles = meta["wtiles"][w]
                    pagg = ps.tile([128, H], FP32, tag="mm", space="PSUM")
                    for j, et in enumerate(tiles):
                        nc.tensor.matmul(
                            pagg[:],
                            lhsT=S[:, (et - ct0) * 128:(et - ct0 + 1) * 128],
                            rhs=mga[:, (et - ct0) * H:(et - ct0 + 1) * H],
                            start=(j == 0), stop=(j == len(tiles) - 1))
                    agg_bf = sb.tile([128, H], BF16, tag="aggbf")
                    if li == 0:
                        # BN fixup on raw aggregate: sc*agg + sdeg (x) sh
                        t2 = sb.tile([128, H], FP32, tag="sq")
                        nc.vector.tensor_tensor(
                            out=t2[:],
                            in0=sdeg_sb[:, w:w + 1].to_broadcast([128, H]),
                            in1=srow[:, H:], op=mybir.AluOpType.mult)
                        t3 = sb.tile([128, H], FP32, tag="sq")
                        nc.vector.tensor_tensor(
                            out=t3[:], in0=pagg[:], in1=srow[:, :H],
                            op=mybir.AluOpType.mult)
                        nc.vector.tensor_add(agg_bf[:], t3[:], t2[:])
                    else:
                        nc.vector.tensor_copy(out=agg_bf[:], in_=pagg[:])
                    pt1 = pst.tile([128, 128], BF16, tag="tp", space="PSUM")
                    nc.tensor.transpose(out=pt1[:], in_=agg_bf[:, :128],
                                        identity=ident[:])
                    pt2 = pst.tile([128, 128], BF16, tag="tp", space="PSUM")
                    nc.tensor.transpose(out=pt2[:], in_=agg_bf[:, 128:],
                                        identity=ident[:])
                    aggT = sb.tile([128, H], BF16, tag="aggT")
                    nc.vector.tensor_copy(out=aggT[:, :128], in_=pt1[:])
                    nc.vector.tensor_copy(out=aggT[:, 128:], in_=pt2[:])
                    pupd = ps.tile([128, H], FP32, tag="mm", space="PSUM")
                    nc.tensor.matmul(pupd[:], lhsT=aggT[:, :128],
                                     rhs=wp_sb[:, :H], start=True, stop=False)
                    nc.tensor.matmul(pupd[:], lhsT=aggT[:, 128:],
                                     rhs=wp_sb[:, H:], start=False, stop=True)
                    # u = pupd - h*extw - src_b*h0
                    wsl = slice(w * H, (w + 1) * H)
                    usl = slice(wi * H, (wi + 1) * H)
                    t1 = sb.tile([128, H], FP32, tag="sq")
                    if li == 0:
                        # h0 == h here: u = pupd - h*(extw + src_b)
                        nc.vector.tensor_mul(t1[:], h_sb[:, wsl], extw0_sb[:])
                        nc.vector.tensor_sub(u_ch[:, usl], pupd[:], t1[:])
                    else:
                        h0w = chp.tile([128, H], BF16, tag="h0c")
                        nc.sync.dma_start(
                            out=h0w[:], in_=h0d_t.ap()[:, w * H:(w + 1) * H])
                        nc.vector.tensor_mul(t1[:], h_sb[:, wsl], extw_sb[:])
                        nc.vector.scalar_tensor_tensor(
                            out=u_ch[:, usl], in0=h0w[:], scalar=-src_b_val,
                            in1=pupd[:], op0=mybir.AluOpType.mult,
                            op1=mybir.AluOpType.add)
                        nc.vector.tensor_sub(u_ch[:, usl], u_ch[:, usl], t1[:])
                # chunk: h += 0.1*elu(u) ; elu = relu(u) - relu(1-exp(u))
                csl = slice(wc * CH * H, (wc + 1) * CH * H)
                e_ch = chp1.tile([128, CH * H], BF16, tag="ech")
                nc.scalar.activation(e_ch[:], u_ch[:],
                                     mybir.ActivationFunctionType.Exp)
                a_ch = chp1.tile([128, CH * H], BF16, tag="ach")
                nc.scalar.activation(a_ch[:], u_ch[:],
                                     mybir.ActivationFunctionType.Relu,
                                     scale=STEP)
                e2_ch = chp1.tile([128, CH * H], BF16, tag="e2ch")
                nc.scalar.activation(e2_ch[:], e_ch[:],
                                     mybir.ActivationFunctionType.Relu,
                                     scale=-1.0, bias=1.0)
                nc.vector.scalar_tensor_tensor(
                    out=a_ch[:], in0=e2_ch[:], scalar=-STEP,
                    in1=a_ch[:], op0=mybir.AluOpType.mult,
                    op1=mybir.AluOpType.add)
                nc.vector.tensor_add(h_sb[:, csl], h_sb[:, csl], a_ch[:])
                # stage each quarter as soon as its windows are final
                if li < _nlayers - 1 and wc in (3, 7, 10):
                    stage_and_allgather(li + 1, {3: 0, 7: 1, 10: 2}[wc])
            if li < _nlayers - 1:
                stage_and_allgather(li + 1, 3)

        # ================= MLP =================
        _mlp_chunks = 0 if os.environ.get("K_NOMLP") else NW // CH
        for wc in range(_mlp_chunks):
            u_ch = chp.tile([128, CH * H], FP32, tag="uch")
            for wi in range(CH):
                w = wc * CH + wi
                wsl = slice(w * H, (w + 1) * H)
                usl = slice(wi * H, (wi + 1) * H)
                pt1 = pst.tile([128, 128], BF16, tag="tp", space="PSUM")
                nc.tensor.transpose(out=pt1[:], in_=h_sb[:, w * H: w * H + 128],
                                    identity=ident[:])
                pt2 = pst.tile([128, 128], BF16, tag="tp", space="PSUM")
                nc.tensor.transpose(out=pt2[:], in_=h_sb[:, w * H + 128: (w + 1) * H],
                                    identity=ident[:])
                hT = sb.tile([128, H], BF16, tag="aggT")
                nc.vector.tensor_copy(out=hT[:, :128], in_=pt1[:])
                nc.vector.tensor_copy(out=hT[:, 128:], in_=pt2[:])
                pg = ps.tile([128, H], FP32, tag="mm", space="PSUM")
                nc.tensor.matmul(pg[:], lhsT=hT[:, :128], rhs=l1_sb[:, :H],
                                 start=True, stop=False)
                nc.tensor.matmul(pg[:], lhsT=hT[:, 128:], rhs=l1_sb[:, H:],
                                 start=False, stop=True)
                nc.vector.tensor_add(u_ch[:, usl], pg[:], b1_sb[:])
            # g = elu(u) = relu(u) - relu(1-exp(u))
            e_ch = chp1.tile([128, CH * H], BF16, tag="ech")
            nc.scalar.activation(e_ch[:], u_ch[:],
                                 mybir.ActivationFunctionType.Exp)
            g_ch = chp1.tile([128, CH * H], BF16, tag="ach")
            nc.scalar.activation(g_ch[:], u_ch[:],
                                 mybir.ActivationFunctionType.Relu)
            e2_ch = chp1.tile([128, CH * H], BF16, tag="e2ch")
            nc.scalar.activation(e2_ch[:], e_ch[:],
                                 mybir.ActivationFunctionType.Relu,
                                 scale=-1.0, bias=1.0)
            nc.vector.tensor_sub(g_ch[:], g_ch[:], e2_ch[:])
            for wi in range(CH):
                w = wc * CH + wi
                sz = 84 if w == NW - 1 else 128
                usl128 = slice(wi * H, wi * H + 128)
                usl256 = slice(wi * H + 128, (wi + 1) * H)
                pt1 = pst.tile([128, 128], BF16, tag="tp", space="PSUM")
                nc.tensor.transpose(out=pt1[:], in_=g_ch[:, usl128],
                                    identity=ident[:])
                pt2 = pst.tile([128, 128], BF16, tag="tp", space="PSUM")
                nc.tensor.transpose(out=pt2[:], in_=g_ch[:, usl256],
                                    identity=ident[:])
                gT = sb.tile([128, H], BF16, tag="aggT")
                nc.vector.tensor_copy(out=gT[:, :128], in_=pt1[:])
                nc.vector.tensor_copy(out=gT[:, 128:], in_=pt2[:])
                py = ps.tile([128, H], FP32, tag="mm", space="PSUM")
                nc.tensor.matmul(py[:], lhsT=gT[:, :128], rhs=l2_sb[:, :H],
                                 start=True, stop=False)
                nc.tensor.matmul(py[:], lhsT=gT[:, 128:], rhs=l2_sb[:, H:],
                                 start=False, stop=True)
                y_sb = sb.tile([128, H], FP32, tag="sq")
                nc.vector.tensor_add(y_sb[:], py[:], b2_sb[:])
                nc.sync.dma_start(out=y_t.ap()[w * 128: w * 128 + sz, :],
                                  in_=y_sb[:sz, :])

    nc.compile()
    return nc


def _host_prep(x, edge_index, enc_w, bn_gamma, bn_beta, ext_w, src_b, pw_W,
               lin1_w, lin1_b, lin2_w, lin2_b):
    x = np.asarray(x, dtype=np.float32)
    ei = np.asarray(edge_index)
    row = ei[0].astype(np.int64)
    col = ei[1].astype(np.int64)

    # pairwise matrix
    pw = np.asarray(pw_W, dtype=np.float32)
    W0 = np.triu(pw[:, :-2], k=1)
    W0 = W0 + W0.T
    Wp = W0 + np.diag(pw[:, -2] * np.abs(W0).sum(1) + pw[:, -1])

    deg = np.bincount(col, minlength=N).astype(np.float32)
    dinv = np.where(deg > 0, deg ** -0.5, 0.0).astype(np.float32)
    nrm = (dinv[row] * dinv[col]).astype(np.float32)

    # ---- e-slot layout: windows x source-quarter, chunked gathers ----
    NB = 4                                 # groups = source quarters
    QW = [(0, 25), (25, 25), (50, 24), (74, 24)]
    qb = np.array([0, 25, 50, 74, 98]) * 128   # node-row boundaries
    core_e = col // NL
    wloc_e = (col % NL) // 128
    dloc_e = (col % NL - wloc_e * 128).astype(np.float32)
    src_core = row // NL
    src_loc = row % NL                     # 0..12499 within source core
    buck_e = np.searchsorted(qb, src_loc, side="right") - 1   # quarter
    qsz = (qb[buck_e + 1] - qb[buck_e])
    rowh_e = src_core * qsz + (src_loc - qb[buck_e])   # row within quarter

    # counts per (core, window, group); tiles shared across cores
    counts = np.zeros((NCORE, NW, NB), dtype=np.int64)
    np.add.at(counts, (core_e, wloc_e, buck_e), 1)
    t_wb = np.ceil(counts.max(0) / 128).astype(np.int64)   # [NW, NB]

    # global tile order: chunk -> group -> window
    NC_CH = NW // CH
    tile_base = np.zeros((NW, NB), dtype=np.int64)
    calls = []           # per chunk: list of (group, n_tiles, col_off, t0)
    chunk_t0 = [0]
    nt_acc = 0
    col_off = 0
    for wc in range(NC_CH):
        ws = range(wc * CH, (wc + 1) * CH)
        ch_calls = []
        for b in range(NB):
            bt0 = nt_acc
            for w in ws:
                tile_base[w, b] = nt_acc
                nt_acc += int(t_wb[w, b])
            nbt = nt_acc - bt0
            # split into dma_gather calls of <= 8 tiles (1024 idx)
            t = bt0
            while t < bt0 + nbt:
                nt = min(8, bt0 + nbt - t)
                ch_calls.append((b, nt, col_off, t))
                col_off += 8 * nt
                t += nt
        calls.append(ch_calls)
        chunk_t0.append(nt_acc)
    n_et = int(nt_acc)
    idx_cols = int(col_off)
    wtiles = [[int(tile_base[w, b]) + t for b in range(NB)
               for t in range(int(t_wb[w, b]))] for w in range(NW)]

    # per-edge slot: order by (core, window, group), rank within group
    gid = (core_e * NW + wloc_e) * NB + buck_e
    order = np.argsort(gid, kind="stable")
    gid_s = gid[order]
    start_of_block = np.zeros(NCORE * NW * NB + 1, dtype=np.int64)
    np.add.at(start_of_block[1:], gid_s, 1)
    start_of_block = np.cumsum(start_of_block)
    rank = np.arange(len(gid_s)) - start_of_block[gid_s]
    tb_flat = tile_base[wloc_e[order], buck_e[order]]
    slot = (tb_flat + rank // 128) * 128 + rank % 128
    core_s = core_e[order]
    rloc_s = rowh_e[order].astype(np.int16)   # idx within quarter (< 25600)
    nrm_s = nrm[order]
    dloc_s = dloc_e[order]

    idxws, sms = [], []
    jj = np.arange(128, dtype=np.float32)
    for c in range(NCORE):
        m = core_s == c
        idx_pad = np.zeros(n_et * 128, dtype=np.int16)
        nrm_pad = np.zeros(n_et * 128, dtype=np.float32)
        dloc_pad = np.full(n_et * 128, -1000.0, dtype=np.float32)
        idx_pad[slot[m]] = rloc_s[m]
        nrm_pad[slot[m]] = nrm_s[m]
        dloc_pad[slot[m]] = dloc_s[m]
        # wrapped int16 indices per call: flat i -> [i%16, i//16], x8 replicate
        idxw = np.zeros((16, idx_cols), dtype=np.int16)
        for ch_calls in calls:
            for (b, nt, co, t0) in ch_calls:
                fl = idx_pad[t0 * 128:(t0 + nt) * 128]
                n = nt * 128
                idxw[np.arange(n) % 16, co + np.arange(n) // 16] = fl
        idxws.append(np.tile(idxw, (8, 1)))
        # host-built one-hot*norm S blocks: sm[p, et*128+j] = S_et[p, j]
        S_all = ((dloc_pad[:, None] == jj[None, :])
                 * nrm_pad[:, None]).astype(bf16)          # [n_et*128, 128]
        sm = np.ascontiguousarray(
            S_all.reshape(n_et, 128, 128).transpose(1, 0, 2)
            .reshape(128, n_et * 128))
        sms.append(sm)

    meta = {"n_et": n_et, "idx_cols": idx_cols, "calls": calls,
            "wtiles": wtiles, "chunk_t0": chunk_t0,
            "src_b": float(np.asarray(src_b).reshape(-1)[0])}

    # per-dest sum of norms (for the layer-0 BN shift fixup)
    sdegs = []
    for c in range(NCORE):
        m = core_e == c
        pad_loc = wloc_e[m] * 128 + (col[m] % NL - wloc_e[m] * 128)
        sd = np.zeros(NW * 128, dtype=np.float32)
        np.add.at(sd, pad_loc, nrm[m])
        sdegs.append(np.ascontiguousarray(sd.reshape(NW, 128).T))

    # per-core padded x^T (bf16)
    xts = []
    for c in range(NCORE):
        xt = np.zeros((FINP, NSH), dtype=bf16)
        xt[:FIN, :NL] = np.ascontiguousarray(
            x[c * NL:(c + 1) * NL].T).astype(bf16)
        xts.append(xt)

    def ktile_layout(mat_T, nk):  # mat_T [nk*128, H] -> [128, nk*H]
        out = np.zeros((128, nk * H), dtype=bf16)
        for kt in range(nk):
            blk = mat_T[kt * 128:(kt + 1) * 128]
            out[:blk.shape[0], kt * H:kt * H + blk.shape[1]] = blk.astype(bf16)
        return out

    enc_wT = np.zeros((FINP, H), dtype=np.float32)
    enc_wT[:FIN] = np.asarray(enc_w, np.float32).T
    encw_h = ktile_layout(enc_wT, KT)
    wp_h = ktile_layout(Wp, 2)                      # symmetric: Wp rows
    l1_h = ktile_layout(np.asarray(lin1_w, np.float32).T, 2)
    l2_h = ktile_layout(np.asarray(lin2_w, np.float32).T, 2)
    extw_h = np.tile(np.asarray(ext_w, np.float32).reshape(1, H), (128, 1))
    b1_h = np.tile(np.asarray(lin1_b, np.float32).reshape(1, H), (128, 1))
    b2_h = np.tile(np.asarray(lin2_b, np.float32).reshape(1, H), (128, 1))
    gb_h = np.concatenate([np.asarray(bn_gamma, np.float32),
                           np.asarray(bn_beta, np.float32)]).reshape(1, 2 * H)

    in_maps = []
    for c in range(NCORE):
        in_maps.append({
            "xt": xts[c], "idxw": idxws[c], "sm": sms[c], "sdeg": sdegs[c],
            "encw": encw_h, "wp": wp_h, "l1": l1_h, "l2": l2_h,
            "extw": extw_h, "b1": b1_h, "b2": b2_h, "gb": gb_h,
        })
    return meta, in_maps


def _run(inputs, trace=False):
    meta, in_maps = _host_prep(**inputs)
    nc = _build_program(meta)
    res = run_bass_kernel_spmd(nc, in_maps, core_ids=list(range(NCORE)),
                               trace=trace)
    y = np.concatenate([res.results[c]["y"] for c in range(NCORE)], 0)
    return y.astype(np.float32), res


def kernel(**inputs):
    y, _ = _run(inputs, trace=False)
    return y


def _timed_run(inputs, n_iter=3):
    """Correctness + warm timing: jit once, device_put inputs, time execs."""
    import time as _time
    import jax
    from jax.sharding import Mesh, PartitionSpec, NamedSharding
    from jax.experimental.shard_map import shard_map
    from concourse import bass2jax, mybir as _mb

    meta, in_maps = _host_prep(**inputs)
    nc = _build_program(meta)
    bass2jax.install_neuronx_cc_hook()

    partition_name = (nc.partition_id_tensor.name
                      if nc.partition_id_tensor else None)
    in_names, out_names, out_avals, zero_outs = [], [], [], []
    for alloc in nc.m.functions[0].allocations:
        if not isinstance(alloc, _mb.MemoryLocationSet):
            continue
        name = alloc.memorylocations[0].name
        if alloc.kind == "ExternalInput":
            if name != partition_name:
                in_names.append(name)
        elif alloc.kind == "ExternalOutput":
            out_names.append(name)
            shape = tuple(alloc.tensor_shape)
            dtype = _mb.dt.np(alloc.dtype)
            out_avals.append(jax.core.ShapedArray(shape, dtype))
            zero_outs.append(np.zeros(shape, dtype))
    n_params = len(in_names)
    n_outs = len(out_avals)
    in_names_all = in_names + out_names
    if partition_name is not None:
        in_names_all.append(partition_name)
    donate = tuple(range(n_params, n_params + n_outs))

    def _body(*args):
        operands = list(args)
        if partition_name is not None:
            operands.append(bass2jax.partition_id_tensor())
        outs = bass2jax._bass_exec_p.bind(
            *operands, out_avals=tuple(out_avals),
            in_names=tuple(in_names_all), out_names=tuple(out_names),
            lowering_input_output_aliases=(),
            sim_require_finite=True, sim_require_nnan=True, nc=nc)
        return tuple(outs)

    devices = jax.devices()[:NCORE]
    mesh = Mesh(np.asarray(devices), ("core",))
    sharded = jax.jit(
        shard_map(_body, mesh=mesh,
                  in_specs=(PartitionSpec("core"),) * (n_params + n_outs),
                  out_specs=(PartitionSpec("core"),) * n_outs,
                  check_rep=False),
        donate_argnums=donate, keep_unused=True)

    sh = NamedSharding(mesh, PartitionSpec("core"))
    concat_in = [
        jax.device_put(
            np.concatenate([np.asarray(in_maps[c][n]) for c in range(NCORE)], 0),
            sh)
        for n in in_names]
    times = []
    out_arrs = None
    for it in range(n_iter):
        concat_zeros = [
            jax.device_put(np.zeros((NCORE * z.shape[0], *z.shape[1:]), z.dtype), sh)
            for z in zero_outs]
        for z in concat_zeros:
            z.block_until_ready()
        t0 = _time.perf_counter()
        out_arrs = sharded(*concat_in, *concat_zeros)
        for o in out_arrs:
            o.block_until_ready()
        times.append(_time.perf_counter() - t0)
    y_full = np.asarray(out_arrs[out_names.index("y")])
    y = y_full.reshape(NCORE, NL, H).reshape(NCORE * NL, H)
    return y.astype(np.float32), times

